# revision 1
# baseline (speedup 1.0000x reference)
"""Self-contained Trainium2 kernel for nn_AutoregressiveGroupQuerySelfAttention.

Reference computation (B=2, S=2048, H=2048, 16 heads x 128 dim):
    q = (x @ Wq.T) -> heads; k likewise; v likewise
    q, k get RoPE; scores = (q @ k.T) * sqrt(D)   (faithful-to-source bug)
    causal softmax; ctx = attn @ v; out = ctx @ Wo.T

Sharding over 8 NeuronCores: core c = (b, g) with b = c // 4 (batch),
g = c % 4 (head-group of 4 heads = 512 hidden columns).  Each core computes
its head-group's context and a partial output  ctx_g @ Wo.T[g-rows, :];
the host sums the 4 partials per batch element.

Precision: the softmax is nearly one-hot (the sqrt(D) score scaling makes
logits ~N(0,128^2)), so the logit path (q/k projections + scores) runs in
fp32r (full-speed reduced-precision fp32 matmul, ~1.5e-4 rel err); v/P/Wo
run in bf16.  Measured end-to-end rel err vs the fp32 reference ~5e-3.
"""
import numpy as np
import ml_dtypes

import concourse.bass as bass
import concourse.mybir as mybir
from concourse import bacc
from concourse.tile import TileContext
from concourse.bass_utils import run_bass_kernel_spmd

F32 = mybir.dt.float32
F32R = mybir.dt.float32r
BF16 = mybir.dt.bfloat16
AX = mybir.AxisListType
ALU = mybir.AluOpType
ACTF = mybir.ActivationFunctionType

B, S, H = 2, 2048, 2048
NUM_HEADS, D = 16, 128
N_CORES = 8
NH = 4                     # heads per core
HG = NH * D                # 512
ROPE_BASE = 10000.0

_NC_CACHE = {}
LAST_RESULTS = None        # BassKernelResults of the most recent run (for profiling)
TRACE = False


def _build(S_=S, H_=H, NH_=NH):
    DD = 128
    HG_ = NH_ * DD
    KT = H_ // 128
    SQT = S_ // 128
    CH = 512
    NCHUNK = S_ // CH

    nc = bacc.Bacc()
    xT = nc.declare_dram_parameter("xT", [H_, S_], F32R, isOutput=False)
    xbfT = nc.declare_dram_parameter("xbfT", [H_, S_], BF16, isOutput=False)
    wqT = nc.declare_dram_parameter("wqT", [H_, HG_], F32R, isOutput=False)
    wkT = nc.declare_dram_parameter("wkT", [H_, HG_], F32R, isOutput=False)
    wvT = nc.declare_dram_parameter("wvT", [H_, HG_], BF16, isOutput=False)
    woT = nc.declare_dram_parameter("woT", [HG_, H_], BF16, isOutput=False)
    cosT = nc.declare_dram_parameter("cosT", [128, S_], F32, isOutput=False)
    sinT = nc.declare_dram_parameter("sinT", [128, S_], F32, isOutput=False)
    rT = nc.declare_dram_parameter("rT", [128, 128], F32R, isOutput=False)
    ident = nc.declare_dram_parameter("ident", [128, 128], BF16, isOutput=False)
    identf = nc.declare_dram_parameter("identf", [128, 128], F32, isOutput=False)
    onesr = nc.declare_dram_parameter("onesr", [1, 128], BF16, isOutput=False)
    mask = nc.declare_dram_parameter("mask", [128, 128], F32, isOutput=False)
    out = nc.declare_dram_parameter("out", [S_, H_], F32, isOutput=True)

    with TileContext(nc) as tc:
        with (
            tc.tile_pool(name="slabs", bufs=1) as slabp,
            tc.tile_pool(name="stats", bufs=3) as statp,
            tc.tile_pool(name="psbig", bufs=4, space="PSUM") as psbig,
            tc.tile_pool(name="pssmall", bufs=3, space="PSUM") as pssmall,
            tc.tile_pool(name="psctx", bufs=1, space="PSUM") as psctx,
        ):
            qrope = [slabp.tile([128, S_], F32R, tag=f"qrope{h}", name=f"qrope{h}") for h in range(NH_)]
            krope = [slabp.tile([128, S_], F32R, tag=f"krope{h}", name=f"krope{h}") for h in range(NH_)]
            vslab = slabp.tile([128, SQT * HG_], BF16, tag="vslab")

            # ====== era 1: q/k projections + RoPE ======
            with (
                tc.tile_pool(name="w1", bufs=1) as wp1,
                tc.tile_pool(name="xin1", bufs=1) as xp1,
                tc.tile_pool(name="tab", bufs=2) as tabp,
                tc.tile_pool(name="work", bufs=2) as workp,
            ):
                rT_sb = wp1.tile([128, 128], F32R, tag="rT")
                nc.sync.dma_start(out=rT_sb[:], in_=rT[:])
                wq_sb = wp1.tile([128, KT * HG_], F32R, tag="wq")
                nc.sync.dma_start(
                    out=wq_sb[:].rearrange("p (kt j) -> p kt j", kt=KT),
                    in_=wqT.rearrange("(kt p) j -> p kt j", p=128),
                )
                wk_sb = wp1.tile([128, KT * HG_], F32R, tag="wk")
                nc.sync.dma_start(
                    out=wk_sb[:].rearrange("p (kt j) -> p kt j", kt=KT),
                    in_=wkT.rearrange("(kt p) j -> p kt j", p=128),
                )

                xT3 = xT.rearrange("(kt p) s -> p kt s", p=128)
                for sc in range(NCHUNK):
                    cs = slice(sc * CH, (sc + 1) * CH)
                    cos_t = tabp.tile([128, CH], F32, tag="cos")
                    nc.sync.dma_start(out=cos_t[:], in_=cosT[:, cs])
                    sin_t = tabp.tile([128, CH], F32, tag="sin")
                    nc.sync.dma_start(out=sin_t[:], in_=sinT[:, cs])
                    xk = []
                    for kt in range(KT):
                        t = xp1.tile([128, CH], F32R, tag=f"xb{kt}", name=f"xb{kt}")
                        nc.sync.dma_start(out=t[:], in_=xT3[:, kt, cs])
                        xk.append(t)
                    pending = None

                    def finish_rope(raw, ropes, h):
                        rotps = pssmall.tile([128, CH], F32, tag="small", name="rotps")
                        nc.tensor.matmul(rotps[:], rT_sb[:], raw[:], start=True, stop=True)
                        t1 = workp.tile([128, CH], F32, tag="t1", name="t1")
                        nc.vector.tensor_mul(t1[:], rotps[:], sin_t[:])
                        t2 = workp.tile([128, CH], F32, tag="t2", name="t2")
                        nc.vector.tensor_mul(t2[:], raw[:].bitcast(F32), cos_t[:])
                        nc.vector.tensor_add(ropes[h][:, cs], t1[:], t2[:])

                    for w_sb, ropes in ((wq_sb, qrope), (wk_sb, krope)):
                        for h in range(NH_):
                            ps = psbig.tile([128, CH], F32, tag="big")
                            for kt in range(KT):
                                nc.tensor.matmul(
                                    ps[:],
                                    w_sb[:, kt * HG_ + h * 128: kt * HG_ + (h + 1) * 128],
                                    xk[kt][:],
                                    start=(kt == 0),
                                    stop=(kt == KT - 1),
                                )
                            raw = workp.tile([128, CH], F32R, tag="raw")
                            nc.vector.tensor_copy(raw[:], ps[:])
                            if pending is not None:
                                finish_rope(*pending)
                            pending = (raw, ropes, h)
                    finish_rope(*pending)

            # ====== era 2: v projection, attention, output projection ======
            with (
                tc.tile_pool(name="w2", bufs=1) as wp2,
                tc.tile_pool(name="xin2", bufs=2) as xp2,
                tc.tile_pool(name="pslab", bufs=3) as pslabp,
                tc.tile_pool(name="ptpool", bufs=2) as ptp,
                tc.tile_pool(name="ctxpool", bufs=1) as ctxp,
                tc.tile_pool(name="ostage", bufs=2) as ostp,
            ):
                ident_sb = wp2.tile([128, 128], BF16, tag="ident")
                nc.sync.dma_start(out=ident_sb[:], in_=ident[:])
                identf_sb = wp2.tile([128, 128], F32, tag="identf")
                nc.sync.dma_start(out=identf_sb[:], in_=identf[:])
                ones_sb = wp2.tile([1, 128], BF16, tag="onesr")
                nc.sync.dma_start(out=ones_sb[:], in_=onesr[:])
                mask_sb = wp2.tile([128, 128], F32, tag="mask")
                nc.sync.dma_start(out=mask_sb[:], in_=mask[:])
                wv_sb = wp2.tile([128, KT * HG_], BF16, tag="wv")
                nc.sync.dma_start(
                    out=wv_sb[:].rearrange("p (kt j) -> p kt j", kt=KT),
                    in_=wvT.rearrange("(kt p) j -> p kt j", p=128),
                )

                xbf3 = xbfT.rearrange("(kt p) s -> p kt s", p=128)
                for t in range(SQT):
                    xv = xp2.tile([128, KT * 128], BF16, tag="xv")
                    nc.sync.dma_start(
                        out=xv[:].rearrange("p (kt s) -> p kt s", kt=KT),
                        in_=xbf3[:, :, t * 128:(t + 1) * 128],
                    )
                    vps = psbig.tile([128, HG_], F32, tag="big")
                    for kt in range(KT):
                        nc.tensor.matmul(
                            vps[:],
                            xv[:, kt * 128:(kt + 1) * 128],
                            wv_sb[:, kt * HG_:(kt + 1) * HG_],
                            start=(kt == 0),
                            stop=(kt == KT - 1),
                        )
                    nc.scalar.copy(vslab[:, t * HG_:(t + 1) * HG_], vps[:])

                wo_sb = wp2.tile([128, NH_ * H_], BF16, tag="wo")
                nc.sync.dma_start(
                    out=wo_sb[:].rearrange("p (j ho) -> p j ho", j=NH_),
                    in_=woT.rearrange("(j p) ho -> p j ho", p=128),
                )

                ctxT = [ctxp.tile([128, S_], BF16, tag=f"ctxT{h}", name=f"ctxT{h}") for h in range(NH_)]
                pt = [ptp.tile([128, CH], BF16, tag=f"pt{t}", name=f"pt{t}") for t in range(SQT)]

                for h in range(NH_):
                    for c in range(NCHUNK):
                        ctxps = psctx.tile([128, CH], F32, tag="ctx")
                        rcp4 = statp.tile([128, 4], F32, tag="rcp4")
                        pend_tr = None

                        def do_transposes(pbf, sq):
                            off = (sq - 4 * c) * 128
                            for t in range(sq + 1):
                                tps = pssmall.tile([128, 128], BF16, tag="small", name="tps")
                                nc.tensor.transpose(
                                    tps[:],
                                    pbf[t // 4][:, (t % 4) * 128:(t % 4 + 1) * 128],
                                    ident_sb[:],
                                )
                                if t % 5 == 4:
                                    nc.scalar.copy(pt[t][:, off:off + 128], tps[:])
                                else:
                                    nc.vector.tensor_copy(pt[t][:, off:off + 128], tps[:])

                        for sq in range(4 * c, 4 * c + 4):
                            nch = sq // 4 + 1
                            ncols = (sq + 1) * 128
                            mx = statp.tile([128, NCHUNK], F32, tag="mx")
                            scps_list = []
                            for kc in range(nch):
                                cols = min(CH, ncols - kc * CH)
                                scps = psbig.tile([128, CH], F32, tag="big")
                                nc.tensor.matmul(
                                    scps[:, :cols],
                                    qrope[h][:, sq * 128:(sq + 1) * 128],
                                    krope[h][:, kc * CH: kc * CH + cols],
                                    start=True,
                                    stop=True,
                                )
                                if kc == nch - 1:
                                    dcol = sq * 128 - kc * CH
                                    nc.vector.tensor_add(
                                        scps[:, dcol:dcol + 128],
                                        scps[:, dcol:dcol + 128],
                                        mask_sb[:],
                                    )
                                if nch > 1:
                                    nc.vector.tensor_reduce(
                                        mx[:, kc:kc + 1], scps[:, :cols], axis=AX.X, op=ALU.max
                                    )
                                scps_list.append((scps, cols))
                            negm = statp.tile([128, 1], F32, tag="negm")
                            if nch == 1:
                                scps0, cols0 = scps_list[0]
                                nc.vector.tensor_reduce(
                                    negm[:], scps0[:, :cols0], axis=AX.X, op=ALU.max, negate=True
                                )
                            else:
                                nc.vector.tensor_reduce(
                                    negm[:], mx[:, :nch], axis=AX.X, op=ALU.max, negate=True
                                )
                            # unnormalized P in bf16; row sums accumulate on ACT
                            pbf = [
                                pslabp.tile([128, CH], BF16, tag=f"pbf{kc}", name=f"pbf{kc}")
                                for kc in range(nch)
                            ]
                            ssum = statp.tile([128, NCHUNK], F32, tag="ssum")
                            for kc, (scps, cols) in enumerate(scps_list):
                                nc.scalar.activation(
                                    pbf[kc][:, :cols],
                                    scps[:, :cols],
                                    ACTF.Exp,
                                    bias=negm[:],
                                    accum_out=ssum[:, kc:kc + 1],
                                )
                            rsum = statp.tile([128, 1], F32, tag="rsum")
                            nc.vector.tensor_reduce(
                                rsum[:], ssum[:, :nch], axis=AX.X, op=ALU.add
                            )
                            nc.vector.reciprocal(rcp4[:, sq - 4 * c: sq - 4 * c + 1], rsum[:])
                            if pend_tr is not None:
                                do_transposes(*pend_tr)
                            pend_tr = (pbf, sq)
                        do_transposes(*pend_tr)
                        tmax = 4 * c + 4
                        for t in range(tmax):
                            c0 = max(0, (t - 4 * c) * 128)
                            nc.tensor.matmul(
                                ctxps[:, c0:CH],
                                vslab[:, t * HG_ + h * 128: t * HG_ + (h + 1) * 128],
                                pt[t][:, c0:CH],
                                start=(t == 0),
                                stop=(t == tmax - 1),
                            )
                        # broadcast the 4 reciprocal-sum columns into a [128, CH] tile
                        rowps = pssmall.tile([1, CH], F32, tag="small")
                        for j in range(4):
                            nc.tensor.transpose(
                                rowps[0:1, j * 128:(j + 1) * 128],
                                rcp4[:, j:j + 1],
                                identf_sb[:],
                            )
                        rrow = statp.tile([1, CH], BF16, tag="rrow")
                        nc.vector.tensor_copy(rrow[:], rowps[:])
                        bcps = pssmall.tile([128, CH], F32, tag="small")
                        nc.tensor.matmul(bcps[:], ones_sb[:], rrow[:], start=True, stop=True)
                        bcsb = statp.tile([128, CH], F32, tag="bcsb")
                        nc.vector.tensor_copy(bcsb[:], bcps[:])
                        nc.vector.tensor_mul(ctxT[h][:, c * CH:(c + 1) * CH], ctxps[:], bcsb[:])

                        if h == NH_ - 1:
                            for st in range(4 * c, 4 * c + 4):
                                ostg = ostp.tile([128, H_], F32, tag="ostg", name="ostg")
                                for hoc in range(H_ // CH):
                                    wops = psbig.tile([128, CH], F32, tag="big", name="wops")
                                    for j in range(NH_):
                                        nc.tensor.matmul(
                                            wops[:],
                                            ctxT[j][:, st * 128:(st + 1) * 128],
                                            wo_sb[:, j * H_ + hoc * CH: j * H_ + (hoc + 1) * CH],
                                            start=(j == 0),
                                            stop=(j == NH_ - 1),
                                        )
                                    nc.scalar.copy(ostg[:, hoc * CH:(hoc + 1) * CH], wops[:])
                                nc.sync.dma_start(out=out[st * 128:(st + 1) * 128, :], in_=ostg[:])




    nc.compile()
    return nc


def _make_tables(S_, D_=128):
    inv_freq = 1.0 / (ROPE_BASE ** (np.arange(0, D_, 2, dtype=np.float32) / D_))
    pos = np.arange(S_, dtype=np.float32)
    ang = pos[:, None] * inv_freq[None, :]
    ang = np.concatenate([ang, ang], axis=1)
    return (
        np.cos(ang).T.astype(np.float32).copy(),
        np.sin(ang).T.astype(np.float32).copy(),
    )


def _make_rot_T(D_=128):
    R = np.zeros((D_, D_), dtype=np.float32)
    half = D_ // 2
    for d in range(half):
        R[d, d + half] = -1.0
    for d in range(half, D_):
        R[d, d - half] = 1.0
    return R.T.copy()


def _make_mask(mask_val=-1e30):
    m = np.zeros((128, 128), dtype=np.float32)
    m[np.triu_indices(128, k=1)] = mask_val
    return m


def kernel(x, Wq, Wk, Wv, Wo):
    """Full inputs in, full output out. Shards over 8 NeuronCores internally."""
    global LAST_RESULTS
    x = np.ascontiguousarray(np.asarray(x, dtype=np.float32))
    Wq = np.asarray(Wq, dtype=np.float32)
    Wk = np.asarray(Wk, dtype=np.float32)
    Wv = np.asarray(Wv, dtype=np.float32)
    Wo = np.asarray(Wo, dtype=np.float32)

    if "nc" not in _NC_CACHE:
        _NC_CACHE["nc"] = _build()
    nc = _NC_CACHE["nc"]

    scale = np.sqrt(np.float32(D))
    cosT, sinT = _make_tables(S)
    rT = _make_rot_T()
    identb = np.eye(128, dtype=ml_dtypes.bfloat16)
    identf = np.eye(128, dtype=np.float32)
    onesr = np.ones((1, 128), dtype=ml_dtypes.bfloat16)
    maskt = _make_mask()

    WqT = Wq.T * scale                    # [H, 16*D], scale folded into q path
    WkT = np.ascontiguousarray(Wk.T)
    WvT_bf = Wv.T.astype(ml_dtypes.bfloat16)
    WoT_bf = Wo.T.astype(ml_dtypes.bfloat16)   # [H(in=ctx), H(out)] rows = ctx hidden

    in_maps = []
    for c in range(N_CORES):
        b, g = divmod(c, NH)
        js = slice(g * HG, (g + 1) * HG)
        xT_b = np.ascontiguousarray(x[b].T)
        in_maps.append({
            "xT": xT_b,
            "xbfT": xT_b.astype(ml_dtypes.bfloat16),
            "wqT": np.ascontiguousarray(WqT[:, js]).astype(np.float32),
            "wkT": np.ascontiguousarray(WkT[:, js]),
            "wvT": np.ascontiguousarray(WvT_bf[:, js]),
            "woT": np.ascontiguousarray(WoT_bf[js, :]),
            "cosT": cosT,
            "sinT": sinT,
            "rT": rT,
            "ident": identb,
            "identf": identf,
            "onesr": onesr,
            "mask": maskt,
        })

    LAST_RESULTS = run_bass_kernel_spmd(
        nc, in_maps, core_ids=list(range(N_CORES)), trace=TRACE
    )
    res = LAST_RESULTS.results

    out = np.zeros((B, S, H), dtype=np.float32)
    for c in range(N_CORES):
        b = c // NH
        out[b] += res[c]["out"]
    return out

